# revision 8
# baseline (speedup 1.0000x reference)
"""Pairwise squared L2 distance (retrieval KNN) on 8 TRN2 NeuronCores.

dist[i, j] = ||x_i||^2 + ||y_j||^2 - 2 * <x_i, y_j>

Sharding: rows of x are split across the 8 cores (data-parallel over n);
y is replicated. Each core computes a [1024, 8192] slab of the distance
matrix.

Memory-roofline design (rel tol 2e-2 allows 16-bit end to end):
- Single fp16 matmul for the cross term (x pre-scaled by -2 host-side,
  so PSUM = -2<x,y>). ~1e-3 max rel err, 20x inside tolerance. Matmuls
  stay single-instruction accumulation groups: split start/stop (e.g.
  folding norm rows via a second accumulate matmul) halves PE issue
  rate on this silicon, so the norms ride the epilogue instead.
- Output stored as fp16 (~17 MB/core instead of 34), host casts back
  to fp32 after the gather. Device HBM traffic ~20 MB/core -> ~57 us
  roofline at 358 GB/s per core.
- Epilogue work (2 passes over 8.4M elems) is spread over three
  engines so none exceeds the DMA floor:
  op1: a = psum + x_sq[p]  (split by column: ScalarE bias-activation
       for cols 0:1728, VectorE tensor_scalar for the rest, so both
       engines drain each PSUM tile concurrently)
  op2: out = a + y_sq[j]   (VectorE fp16 tensor_tensor, 4096 wide;
       GpSimd shares VectorE's SBUF port, so it only builds the late
       half of the ysq broadcast tile)
  The y_sq broadcast tile comes from the host for cols 0:4096 (needed
  in the first ~15 us) and from GpSimd partition_broadcast for cols
  4096:8192 (needed after ~35 us, hiding the ~6 us Q7 library load).
- A dummy ACTIVATE at the top pulls the one-time ~2.7 us ACT table
  load into the DMA load phase.
"""

import numpy as np

import concourse.bass as bass
import concourse.mybir as mybir
import concourse.tile as tile
from concourse import bacc
from concourse.bass import ts
from concourse.bass_utils import run_bass_kernel_spmd

N, M, D = 8192, 8192, 128
NCORES = 8
SLAB = N // NCORES  # 1024 rows of x per core
P = 128  # partitions / m-chunk height
MCH = SLAB // P  # 8 m-chunks per core
NT = 512  # matmul free-dim tile (one fp32 PSUM bank)
GW = 4  # n-chunks per PSUM group (4 banks = 8 KiB/partition)
GCOLS = GW * NT  # 2048
NG = M // GCOLS  # 4 column groups
PCOLS = 2 * GCOLS  # 4096: op2/store width (two groups)
HB = M // 2  # 4096: host-provided half of the ysq broadcast tile

_f32 = mybir.dt.float32
_f16 = mybir.dt.float16
_IDENT = mybir.ActivationFunctionType.Identity

# op1 column split: ScalarE takes cols 0:XS, VectorE takes XS:GCOLS of
# every block, so both engines always drain the live PSUM tile together.
XS = 1728

_compiled_nc = None


def _build():
    """Build + compile the single-core Bass program (SPMD across 8 cores)."""
    nc = bacc.Bacc(
        "TRN2",
        target_bir_lowering=False,
        debug=False,
        enable_asserts=False,
        num_devices=NCORES,
    )
    xh = nc.dram_tensor("xh", [D, SLAB], _f16, kind="ExternalInput").ap()
    yh = nc.dram_tensor("yh", [D, M], _f16, kind="ExternalInput").ap()
    xsq = nc.dram_tensor("xsq", [P, MCH], _f32, kind="ExternalInput").ap()
    ysqb = nc.dram_tensor("ysqb", [P, HB], _f16, kind="ExternalInput").ap()
    ysqr = nc.dram_tensor("ysqr", [1, M - HB], _f16, kind="ExternalInput").ap()
    dist = nc.dram_tensor("dist", [SLAB, M], _f16, kind="ExternalOutput").ap()

    with tile.TileContext(nc) as tc:
        with (
            tc.tile_pool(name="consts", bufs=1) as cpool,
            tc.tile_pool(name="psum", bufs=2, space="PSUM") as pspool,
            tc.tile_pool(name="abuf", bufs=3) as apool,
            tc.tile_pool(name="obuf", bufs=4) as opool,
        ):
            # Warm the ACT spline tables during the load phase.
            dum = cpool.tile([1, 8], _f32)
            nc.vector.memset(dum[:], 0.0)
            dum2 = cpool.tile([1, 8], _f32)
            nc.scalar.activation(dum2[:], dum[:], _IDENT, bias=0.0, scale=1.0)

            # First-block inputs lead so the PE can start ASAP.
            xh_sb = cpool.tile([D, SLAB], _f16)
            nc.sync.dma_start(xh_sb[:], xh[:])
            yh_sb = cpool.tile([D, M], _f16)
            nc.sync.dma_start(yh_sb[:, 0:NT], yh[:, 0:NT])
            nc.sync.dma_start(yh_sb[:, NT:GCOLS], yh[:, NT:GCOLS])
            nc.sync.dma_start(yh_sb[:, ts(1, GCOLS)], yh[:, ts(1, GCOLS)])
            ysq_b = cpool.tile([P, M], _f16)
            nc.sync.dma_start(ysq_b[:, 0:HB], ysqb[:])
            xsq_sb = cpool.tile([P, MCH], _f32)
            nc.sync.dma_start(xsq_sb[:], xsq[:])
            ysqr_sb = cpool.tile([1, M - HB], _f16)
            nc.sync.dma_start(ysqr_sb[:], ysqr[:])
            nc.sync.dma_start(yh_sb[:, ts(2, GCOLS)], yh[:, ts(2, GCOLS)])
            nc.sync.dma_start(yh_sb[:, ts(3, GCOLS)], yh[:, ts(3, GCOLS)])

            # ysq_b[p, j] = y_sq[j] for the back half, built on GpSimd
            # (its ~6 us library load hides behind the first column pairs).
            for c in range(2):
                nc.gpsimd.partition_broadcast(
                    ysq_b[:, HB + c * GCOLS : HB + (c + 1) * GCOLS],
                    ysqr_sb[0:1, ts(c, GCOLS)],
                )

            blk = 0
            for gp in range(NG // 2):
                for mc in range(MCH):
                    xh_w = xh_sb[:, ts(mc, P)]
                    xsq_col = xsq_sb[:, mc : mc + 1]
                    a4 = apool.tile([P, PCOLS], _f16, tag="a")
                    for half in range(2):
                        g = 2 * gp + half
                        ps = pspool.tile([P, GCOLS], _f32, tag="ps")
                        for jj in range(GW):
                            nc.tensor.matmul(
                                ps[:, ts(jj, NT)],
                                xh_w,
                                yh_sb[:, ts(g * GW + jj, NT)],
                                start=True,
                                stop=True,
                            )
                        # op1: a = psum + x_sq (per-partition), split
                        # across ScalarE and VectorE so they overlap.
                        ah = a4[:, half * GCOLS : (half + 1) * GCOLS]
                        nc.scalar.activation(
                            ah[:, 0:XS], ps[:, 0:XS], _IDENT,
                            bias=xsq_col, scale=1.0,
                        )
                        nc.vector.tensor_scalar_add(
                            ah[:, XS:GCOLS], ps[:, XS:GCOLS], xsq_col
                        )
                        blk += 1
                    # op2: out = a + y_sq over both groups at once
                    ot = opool.tile([P, PCOLS], _f16, tag="ot")
                    nc.vector.tensor_add(ot[:], a4[:], ysq_b[:, ts(gp, PCOLS)])
                    nc.sync.dma_start(dist[ts(mc, P), ts(gp, PCOLS)], ot[:])

    nc.compile()
    return nc


def _get_nc():
    global _compiled_nc
    if _compiled_nc is None:
        _compiled_nc = _build()
    return _compiled_nc


def make_in_maps(x: np.ndarray, y: np.ndarray) -> list[dict[str, np.ndarray]]:
    x = np.asarray(x, dtype=np.float32)
    y = np.asarray(y, dtype=np.float32)
    x_sq = np.sum(x * x, axis=1, dtype=np.float32)
    y_sq = np.sum(y * y, axis=1, dtype=np.float32)

    xt2 = np.ascontiguousarray((-2.0 * x).T.astype(np.float16))  # [D, N]
    yt = np.ascontiguousarray(y.T.astype(np.float16))  # [D, M]
    ysq16 = y_sq.astype(np.float16)
    ysqb = np.ascontiguousarray(np.broadcast_to(ysq16[:HB], (P, HB)))
    ysqr = np.ascontiguousarray(ysq16[HB:].reshape(1, M - HB))

    in_maps = []
    for c in range(NCORES):
        sl = slice(c * SLAB, (c + 1) * SLAB)
        # [P, MCH]: column mc holds x_sq for rows mc*128..mc*128+127
        xsq_in = np.ascontiguousarray(x_sq[sl].reshape(MCH, P).T)
        in_maps.append(
            {
                "xh": np.ascontiguousarray(xt2[:, sl]),
                "yh": yt,
                "xsq": xsq_in,
                "ysqb": ysqb,
                "ysqr": ysqr,
            }
        )
    return in_maps


def kernel(x: np.ndarray, y: np.ndarray, **run_kwargs) -> np.ndarray:
    nc = _get_nc()
    in_maps = make_in_maps(x, y)
    res = run_bass_kernel_spmd(nc, in_maps, core_ids=list(range(NCORES)), **run_kwargs)
    out = np.concatenate(
        [res.results[c]["dist"] for c in range(NCORES)], axis=0
    ).astype(np.float32)
    if run_kwargs:
        kernel.last_results = res
    return out


# revision 9
# speedup vs baseline: 1.1027x; 1.1027x over previous
"""Pairwise squared L2 distance (retrieval KNN) on 8 TRN2 NeuronCores.

dist[i, j] = ||x_i||^2 + ||y_j||^2 - 2 * <x_i, y_j>

Sharding: rows of x are split across the 8 cores (data-parallel over n);
y is replicated. Each core computes a [1024, 8192] slab of the distance
matrix.

Memory-roofline design (rel tol 2e-2 allows 16-bit end to end):
- Single fp16 matmul for the cross term (x pre-scaled by -2 host-side,
  so PSUM = -2<x,y>). ~1e-3 max rel err, 20x inside tolerance. Matmuls
  stay single-instruction accumulation groups: splitting start/stop to
  fold the norms in via a second accumulate matmul halves PE issue
  rate on this silicon, so the norms ride the epilogue instead.
- Output stored as fp16 (~17 MB/core instead of 34), host casts back to
  fp32 after the gather. Device HBM traffic ~19 MB/core -> ~54 us
  roofline at 358 GB/s per core.
- Epilogue split across engines so neither exceeds the DMA floor:
  op1: a = psum + x_sq[p]   (per-partition bias; ScalarE for 26
       blocks at ~2.0 us/block, VectorE tensor_scalar for 6)
  op2: out = a + y_sq[j]    (VectorE fp16 tensor_tensor, 2x mode,
       ~1.5 us/block; y_sq broadcast tile built once by GpSimd, whose
       shared SBUF port makes it unsuitable for more than that)
- A dummy ACTIVATE at the top pulls the one-time ~2.7 us ACT table
  load into the DMA load phase.
"""

import numpy as np

import concourse.bass as bass
import concourse.mybir as mybir
import concourse.tile as tile
from concourse import bacc
from concourse.bass import ts
from concourse.bass_utils import run_bass_kernel_spmd

N, M, D = 8192, 8192, 128
NCORES = 8
SLAB = N // NCORES  # 1024 rows of x per core
P = 128  # partitions / m-chunk height
MCH = SLAB // P  # 8 m-chunks per core
NT = 512  # matmul free-dim tile (one fp32 PSUM bank)
GW = 4  # n-chunks per PSUM group (4 banks = 8 KiB/partition)
GCOLS = GW * NT  # 2048
NG = M // GCOLS  # 4 column groups
LW = 2048  # y load-chunk width
YC = M // LW  # 4 load chunks

_f32 = mybir.dt.float32
_f16 = mybir.dt.float16
_IDENT = mybir.ActivationFunctionType.Identity

# Blocks (of 32) whose op1 runs on VectorE instead of ScalarE, to keep
# ScalarE under the DMA roofline. 6/32 balances ACT ~51us, DVE ~53us.
_DVE_OP1 = {4, 9, 14, 19, 24, 29}

_compiled_nc = None


def _build():
    """Build + compile the single-core Bass program (SPMD across 8 cores)."""
    nc = bacc.Bacc(
        "TRN2",
        target_bir_lowering=False,
        debug=False,
        enable_asserts=False,
        num_devices=NCORES,
    )
    xh = nc.dram_tensor("xh", [D, SLAB], _f16, kind="ExternalInput").ap()
    yh = nc.dram_tensor("yh", [D, M], _f16, kind="ExternalInput").ap()
    xsq = nc.dram_tensor("xsq", [P, MCH], _f32, kind="ExternalInput").ap()
    ysq = nc.dram_tensor("ysq", [1, M], _f16, kind="ExternalInput").ap()
    dist = nc.dram_tensor("dist", [SLAB, M], _f16, kind="ExternalOutput").ap()

    with tile.TileContext(nc) as tc:
        with (
            tc.tile_pool(name="consts", bufs=1) as cpool,
            tc.tile_pool(name="psum", bufs=2, space="PSUM") as pspool,
            tc.tile_pool(name="abuf", bufs=4) as apool,
            tc.tile_pool(name="obuf", bufs=4) as opool,
        ):
            # Warm the ACT spline tables during the load phase.
            dum = cpool.tile([1, 8], _f32)
            nc.vector.memset(dum[:], 0.0)
            dum2 = cpool.tile([1, 8], _f32)
            nc.scalar.activation(dum2[:], dum[:], _IDENT, bias=0.0, scale=1.0)

            # First-group inputs lead so the PE can start ASAP: the ysq row
            # (gates the GpSimd broadcasts), the first y columns, then x.
            ysq_row = cpool.tile([1, M], _f16)
            nc.sync.dma_start(ysq_row[:], ysq[:])
            yh_sb = cpool.tile([D, M], _f16)
            nc.sync.dma_start(yh_sb[:, 0:NT], yh[:, 0:NT])
            xh_sb = cpool.tile([D, SLAB], _f16)
            nc.sync.dma_start(xh_sb[:], xh[:])
            nc.sync.dma_start(yh_sb[:, NT:LW], yh[:, NT:LW])
            xsq_sb = cpool.tile([P, MCH], _f32)
            nc.sync.dma_start(xsq_sb[:], xsq[:])
            for c in range(1, YC):
                nc.sync.dma_start(yh_sb[:, ts(c, LW)], yh[:, ts(c, LW)])

            # ysq_b[p, j] = y_sq[j] (fp16), built on the otherwise-idle
            # GpSimd engine in group-sized chunks.
            ysq_b = cpool.tile([P, M], _f16)
            for c in range(YC):
                nc.gpsimd.partition_broadcast(
                    ysq_b[:, ts(c, LW)], ysq_row[0:1, ts(c, LW)]
                )

            def emit_block(blk, mc, g):
                """One [128, 2048] output block: 4 matmuls + epilogue + store."""
                xh_w = xh_sb[:, ts(mc, P)]
                xsq_col = xsq_sb[:, mc : mc + 1]
                ps = pspool.tile([P, GCOLS], _f32, tag="ps")
                for jj in range(GW):
                    nc.tensor.matmul(
                        ps[:, ts(jj, NT)],
                        xh_w,
                        yh_sb[:, ts(g * GW + jj, NT)],
                        start=True,
                        stop=True,
                    )
                # op1: a = psum + x_sq (per-partition)
                a = apool.tile([P, GCOLS], _f16, tag="a")
                if blk in _DVE_OP1:
                    nc.vector.tensor_scalar_add(a[:], ps[:], xsq_col)
                else:
                    nc.scalar.activation(
                        a[:], ps[:], _IDENT, bias=xsq_col, scale=1.0
                    )
                # op2: out = a + y_sq (fp16 2x mode on VectorE)
                ot = opool.tile([P, GCOLS], _f16, tag="ot")
                nc.vector.tensor_add(ot[:], a[:], ysq_b[:, ts(g, GCOLS)])
                nc.sync.dma_start(dist[ts(mc, P), ts(g, GCOLS)], ot[:])

            blk = 0
            for g in range(NG):
                for mc in range(MCH):
                    emit_block(blk, mc, g)
                    blk += 1

    nc.compile()
    return nc


def _get_nc():
    global _compiled_nc
    if _compiled_nc is None:
        _compiled_nc = _build()
    return _compiled_nc


def make_in_maps(x: np.ndarray, y: np.ndarray) -> list[dict[str, np.ndarray]]:
    x = np.asarray(x, dtype=np.float32)
    y = np.asarray(y, dtype=np.float32)
    x_sq = np.sum(x * x, axis=1, dtype=np.float32)
    y_sq = np.sum(y * y, axis=1, dtype=np.float32)

    xt2 = np.ascontiguousarray((-2.0 * x).T.astype(np.float16))  # [D, N]
    yt = np.ascontiguousarray(y.T.astype(np.float16))  # [D, M]
    ysq_in = np.ascontiguousarray(y_sq.astype(np.float16).reshape(1, M))

    in_maps = []
    for c in range(NCORES):
        sl = slice(c * SLAB, (c + 1) * SLAB)
        # [P, MCH]: column mc holds x_sq for rows mc*128..mc*128+127
        xsq_in = np.ascontiguousarray(x_sq[sl].reshape(MCH, P).T)
        in_maps.append(
            {
                "xh": np.ascontiguousarray(xt2[:, sl]),
                "yh": yt,
                "xsq": xsq_in,
                "ysq": ysq_in,
            }
        )
    return in_maps


def kernel(x: np.ndarray, y: np.ndarray, **run_kwargs) -> np.ndarray:
    nc = _get_nc()
    in_maps = make_in_maps(x, y)
    res = run_bass_kernel_spmd(nc, in_maps, core_ids=list(range(NCORES)), **run_kwargs)
    out = np.concatenate(
        [res.results[c]["dist"] for c in range(NCORES)], axis=0
    ).astype(np.float32)
    if run_kwargs:
        kernel.last_results = res
    return out


# revision 10
# speedup vs baseline: 1.1068x; 1.0036x over previous
"""Pairwise squared L2 distance (retrieval KNN) on 8 TRN2 NeuronCores.

dist[i, j] = ||x_i||^2 + ||y_j||^2 - 2 * <x_i, y_j>

Sharding: rows of x are split across the 8 cores (data-parallel over n);
y is replicated. Each core computes a [1024, 8192] slab of the distance
matrix.

Memory-roofline design (rel tol 2e-2 allows 16-bit end to end):
- Single fp16 matmul for the cross term (x pre-scaled by -2 host-side,
  so PSUM = -2<x,y>). ~1e-3 max rel err, 20x inside tolerance. Matmuls
  stay single-instruction accumulation groups: splitting start/stop to
  fold the norms in via a second accumulate matmul halves PE issue
  rate on this silicon, so the norms ride the epilogue instead.
- Output stored as fp16 (~17 MB/core instead of 34), host casts back to
  fp32 after the gather. Device HBM traffic ~19 MB/core -> ~54 us
  roofline at 358 GB/s per core.
- Epilogue split across engines so neither exceeds the DMA floor:
  op1: a = psum + x_sq[p]   (per-partition bias; ScalarE for 26
       blocks at ~2.0 us/block, VectorE tensor_scalar for 6)
  op2: out = a + y_sq[j]    (VectorE fp16 tensor_tensor, 2x mode,
       ~1.5 us/block; y_sq broadcast tile built once by GpSimd, whose
       shared SBUF port makes it unsuitable for more than that)
- A dummy ACTIVATE at the top pulls the one-time ~2.7 us ACT table
  load into the DMA load phase.
"""

import numpy as np

import concourse.bass as bass
import concourse.mybir as mybir
import concourse.tile as tile
from concourse import bacc
from concourse.bass import ts
from concourse.bass_utils import run_bass_kernel_spmd

N, M, D = 8192, 8192, 128
NCORES = 8
SLAB = N // NCORES  # 1024 rows of x per core
P = 128  # partitions / m-chunk height
MCH = SLAB // P  # 8 m-chunks per core
NT = 512  # matmul free-dim tile (one fp32 PSUM bank)
GW = 4  # n-chunks per PSUM group (4 banks = 8 KiB/partition)
GCOLS = GW * NT  # 2048
NG = M // GCOLS  # 4 column groups
LW = 2048  # y load-chunk width
YC = M // LW  # 4 load chunks

_f32 = mybir.dt.float32
_f16 = mybir.dt.float16
_IDENT = mybir.ActivationFunctionType.Identity

# Blocks (of 32) whose op1 runs on VectorE instead of ScalarE, to keep
# ScalarE under the DMA roofline. 6/32 balances ACT ~51us, DVE ~53us.
_DVE_OP1 = {4, 9, 14, 19, 24, 29}

_compiled_nc = None


def _build():
    """Build + compile the single-core Bass program (SPMD across 8 cores)."""
    nc = bacc.Bacc(
        "TRN2",
        target_bir_lowering=False,
        debug=False,
        enable_asserts=False,
        num_devices=NCORES,
    )
    xh = nc.dram_tensor("xh", [D, SLAB], _f16, kind="ExternalInput").ap()
    yh = nc.dram_tensor("yh", [D, M], _f16, kind="ExternalInput").ap()
    xsq = nc.dram_tensor("xsq", [P, MCH], _f32, kind="ExternalInput").ap()
    ysq = nc.dram_tensor("ysq", [1, M], _f16, kind="ExternalInput").ap()
    dist = nc.dram_tensor("dist", [SLAB, M], _f16, kind="ExternalOutput").ap()

    with tile.TileContext(nc) as tc:
        with (
            tc.tile_pool(name="consts", bufs=1) as cpool,
            tc.tile_pool(name="psum", bufs=2, space="PSUM") as pspool,
            tc.tile_pool(name="abuf", bufs=8) as apool,
            tc.tile_pool(name="obuf", bufs=8) as opool,
        ):
            # Warm the ACT spline tables during the load phase.
            dum = cpool.tile([1, 8], _f32)
            nc.vector.memset(dum[:], 0.0)
            dum2 = cpool.tile([1, 8], _f32)
            nc.scalar.activation(dum2[:], dum[:], _IDENT, bias=0.0, scale=1.0)

            # First-group inputs lead so the PE can start ASAP: the ysq row
            # (gates the GpSimd broadcasts), the first y columns, then x.
            ysq_row = cpool.tile([1, M], _f16)
            nc.sync.dma_start(ysq_row[:], ysq[:])
            yh_sb = cpool.tile([D, M], _f16)
            nc.sync.dma_start(yh_sb[:, 0:NT], yh[:, 0:NT])
            xh_sb = cpool.tile([D, SLAB], _f16)
            nc.sync.dma_start(xh_sb[:], xh[:])
            nc.sync.dma_start(yh_sb[:, NT:LW], yh[:, NT:LW])
            xsq_sb = cpool.tile([P, MCH], _f32)
            nc.sync.dma_start(xsq_sb[:], xsq[:])
            for c in range(1, YC):
                nc.sync.dma_start(yh_sb[:, ts(c, LW)], yh[:, ts(c, LW)])

            # ysq_b[p, j] = y_sq[j] (fp16), built on the otherwise-idle
            # GpSimd engine in group-sized chunks.
            ysq_b = cpool.tile([P, M], _f16)
            for c in range(YC):
                nc.gpsimd.partition_broadcast(
                    ysq_b[:, ts(c, LW)], ysq_row[0:1, ts(c, LW)]
                )

            def emit_block(blk, mc, g):
                """One [128, 2048] output block: 4 matmuls + epilogue + store."""
                xh_w = xh_sb[:, ts(mc, P)]
                xsq_col = xsq_sb[:, mc : mc + 1]
                ps = pspool.tile([P, GCOLS], _f32, tag="ps")
                for jj in range(GW):
                    nc.tensor.matmul(
                        ps[:, ts(jj, NT)],
                        xh_w,
                        yh_sb[:, ts(g * GW + jj, NT)],
                        start=True,
                        stop=True,
                    )
                # op1: a = psum + x_sq (per-partition)
                a = apool.tile([P, GCOLS], _f16, tag="a")
                if blk in _DVE_OP1:
                    nc.vector.tensor_scalar_add(a[:], ps[:], xsq_col)
                else:
                    nc.scalar.activation(
                        a[:], ps[:], _IDENT, bias=xsq_col, scale=1.0
                    )
                # op2: out = a + y_sq (fp16 2x mode on VectorE)
                ot = opool.tile([P, GCOLS], _f16, tag="ot")
                nc.vector.tensor_add(ot[:], a[:], ysq_b[:, ts(g, GCOLS)])
                nc.sync.dma_start(dist[ts(mc, P), ts(g, GCOLS)], ot[:])

            blk = 0
            for g in range(NG):
                for mc in range(MCH):
                    emit_block(blk, mc, g)
                    blk += 1

    nc.compile()
    return nc


def _get_nc():
    global _compiled_nc
    if _compiled_nc is None:
        _compiled_nc = _build()
    return _compiled_nc


def make_in_maps(x: np.ndarray, y: np.ndarray) -> list[dict[str, np.ndarray]]:
    x = np.asarray(x, dtype=np.float32)
    y = np.asarray(y, dtype=np.float32)
    x_sq = np.sum(x * x, axis=1, dtype=np.float32)
    y_sq = np.sum(y * y, axis=1, dtype=np.float32)

    xt2 = np.ascontiguousarray((-2.0 * x).T.astype(np.float16))  # [D, N]
    yt = np.ascontiguousarray(y.T.astype(np.float16))  # [D, M]
    ysq_in = np.ascontiguousarray(y_sq.astype(np.float16).reshape(1, M))

    in_maps = []
    for c in range(NCORES):
        sl = slice(c * SLAB, (c + 1) * SLAB)
        # [P, MCH]: column mc holds x_sq for rows mc*128..mc*128+127
        xsq_in = np.ascontiguousarray(x_sq[sl].reshape(MCH, P).T)
        in_maps.append(
            {
                "xh": np.ascontiguousarray(xt2[:, sl]),
                "yh": yt,
                "xsq": xsq_in,
                "ysq": ysq_in,
            }
        )
    return in_maps


def kernel(x: np.ndarray, y: np.ndarray, **run_kwargs) -> np.ndarray:
    nc = _get_nc()
    in_maps = make_in_maps(x, y)
    res = run_bass_kernel_spmd(nc, in_maps, core_ids=list(range(NCORES)), **run_kwargs)
    out = np.concatenate(
        [res.results[c]["dist"] for c in range(NCORES)], axis=0
    ).astype(np.float32)
    if run_kwargs:
        kernel.last_results = res
    return out
